# revision 46
# baseline (speedup 1.0000x reference)
"""Multi-head causal attention (B=4, S=2048, H=16, Dh=64, Dm=1024) on 8
Trainium2 NeuronCores.

Sharding: core c handles batch b = c//2 and heads [8*(c%2), 8*(c%2)+8).
Each core computes its 8 heads' full attention + O-projection partial sum;
the host adds the two half-head partials per batch plus O_b.

v3 layout (all matmul inputs bf16, PSUM f32):
  - i-outer / pair-inner block order: for each q-block i, the four head
    pairs run back to back, then the O-projection units for block i
    become fillers for the i+1 phase.  This spreads the O-projection PE
    work and the y DMA across the whole timeline (v2 backloaded both).
  - Loads are single blob descriptors (per-descriptor cost ~2.6us for
    128 strided rows): wv halves lead the two HWDGE rings, then the x
    pair-blobs, then wo/wk; the slow SWDGE ring carries wq + constants.
    v0..3 run as m-halves over four concurrent PSUM groups (2 proj + 2
    borrowed ev bufs) so their first matmuls start when the first x
    blob pair lands.
  - Cross-block priming: each block emits the next block's first two
    logits tiles in its own tail, so the next block's exp stream is
    already running at the boundary.
  - Softmax: exp (ACT) is the only steady-state Scalar work besides the
    per-block Ln/Exp reciprocal (same ACT table set, no reloads).
    Denominators come from the ones-column of the v tiles (M=65 S@V);
    the reciprocal row is replicated across 64 partitions with a PE
    broadcast matmul (one64 x rec) and applied with two DVE multiplies;
    head B is repacked to partitions 64:127 with one SBUF->SBUF DMA.
  - O-projection PSUM is evacuated to bf16 yt tiles on DVE and DMA'd to
    y (bf16, halves the output traffic); the host sums the two per-batch
    partials in f32.  The last phase emits per-dm half units for a
    leaner drain, and the last block's chain runs undeferred.
  - Causal narrowing: for diagonal k-tiles only columns >= o are computed
    (logits matmul, exp, S@V); the 128-wide staircase strip is masked with
    one DVE multiply.
"""

import os
import sys

sys.path.insert(0, "/opt/trn_rl_repo")

import numpy as np

B, S, DM, H, DH = 4, 2048, 1024, 16, 64
HPC = 8          # heads per core
NPAIR = HPC // 2
PB = 512         # q block width
NQP = S // PB    # 4 q blocks
MT = DM // 128   # 8 m-tiles
NKT = S // 128   # 16 k tiles

_cache = {}


def _split_multi_waits(nc, mybir):
    # This container's walrus rejects >1 sync wait per instruction
    # ("Too many sync wait commands").  Move extra waits onto same-engine
    # NoOps right before the instruction; per-engine program order makes
    # this equivalent.
    ctr = 0
    for fn in nc.m.functions:
        for blk in fn.blocks:
            insts = list(blk.instructions)
            new_insts = []
            changed = False
            for inst in insts:
                si = getattr(inst, "sync_info", None)
                waits = list(si.on_wait) if (si is not None and si.on_wait) else []
                if len(waits) > 1:
                    changed = True
                    for w in waits[:-1]:
                        ctr += 1
                        new_insts.append(
                            mybir.InstNoOp(
                                name=f"waitsplit-{ctr}",
                                engine=inst.engine,
                                ins=[],
                                outs=[],
                                sync_info=mybir.SyncInfo(on_wait=[w], on_update=[]),
                            )
                        )
                    si.on_wait = [waits[-1]]
                new_insts.append(inst)
            if changed:
                blk.instructions = new_insts


def _patch_tile_drain(tile_mod, bass_mod):
    # Same walrus limitation hits the Tile kernel-tail drain (one wait per
    # ticked proc).  Chain the waits through single-wait sync NoOps.
    from concourse.vector_clock import ScopedClock, VectorClock

    def _drain_and_barrier(self, tick_clock, wait_clock):
        gc = tick_clock.global_clock
        n = len(gc)
        ticks = [gc[i] for i in range(n)]
        for p in [i for i in range(n) if ticks[i] > 0]:
            nop = self.nc.sync.nop(nofuse=True, hint="drain_wait_split")
            vc = VectorClock([ticks[j] if j == p else 0 for j in range(n)])
            wait_clock.add_sem_waits(nop.ins, ScopedClock({None: vc}))
        self.nc.sync.drain()
        self.nc.all_engine_barrier()
        assert self.sems is not None
        popped = self.nc._tile_sem_poison_stack.pop()
        assert popped is self._sem_poison
        self.nc.clear_and_free_semaphores(list(self.sems.allocated().values()))
        self.nc.all_engine_barrier()

    tile_mod.TileContext._drain_and_barrier = _drain_and_barrier


def _build():
    if "nc" in _cache:
        return _cache["nc"]

    import concourse.bass as bass
    import concourse.mybir as mybir
    import concourse.tile as tile
    from concourse import library_config

    _patch_tile_drain(tile, bass)

    f32 = mybir.dt.float32
    f32r = mybir.dt.float32r
    bf16 = mybir.dt.bfloat16
    Exp = mybir.ActivationFunctionType.Exp
    Ln = mybir.ActivationFunctionType.Ln

    nc = bass.Bass()
    xP = [nc.dram_tensor(f"xP{g}", [128, 2 * S], bf16, kind="ExternalInput")
          for g in range(MT // 2)]
    Wq = nc.dram_tensor("Wq", [128, MT * 512], bf16, kind="ExternalInput")
    Wk = nc.dram_tensor("Wk", [128, MT * 512], bf16, kind="ExternalInput")
    Wv = nc.dram_tensor("Wv", [128, MT * 512], bf16, kind="ExternalInput")
    Wo = nc.dram_tensor("Wo", [128, 4 * DM], bf16, kind="ExternalInput")
    qkb = nc.dram_tensor("qkb", [128, 8], f32, kind="ExternalInput")
    vbb = nc.dram_tensor("vbb", [128, 512], bf16, kind="ExternalInput")
    stair2 = nc.dram_tensor("stair2", [128, 256], bf16, kind="ExternalInput")
    onz = nc.dram_tensor("onz", [128, 8], bf16, kind="ExternalInput")
    one64 = nc.dram_tensor("one64", [1, 64], f32r, kind="ExternalInput")
    y = nc.dram_tensor("y", [S, DM], bf16, kind="ExternalOutput")

    with tile.TileContext(nc) as tc:
        with nc.allow_low_precision(reason="bf16 tiles feeding the PE"), \
             tc.tile_pool(name="mp", bufs=1) as mp, \
             tc.tile_pool(name="sp", bufs=1) as sp, \
             tc.tile_pool(name="pp", bufs=1, space="PSUM") as pp:

            # ---- input loads ----
            # Each load is ONE blob descriptor (per-descriptor cost is
            # ~2-3us for 128 strided partition rows, so count matters).
            # The HWDGE rings (sync/scalar, ~115GB/s) carry wv halves
            # first, then the x pair-blobs, then wo/wk; the slow gpsimd
            # SWDGE ring (~45GB/s) carries only wq + small constants.
            wv_sb = mp.tile([128, MT * 512], bf16, tag="wv_sb")
            nc.sync.dma_start(wv_sb[:, 0:MT * 256], Wv[:, 0:MT * 256])
            nc.scalar.dma_start(wv_sb[:, MT * 256:], Wv[:, MT * 256:])
            xp = [mp.tile([128, 2 * S], bf16, tag=f"xp{g}", name=f"xp{g}")
                  for g in range(MT // 2)]
            for g in range(MT // 2):
                eng = nc.sync if g % 2 == 0 else nc.scalar
                eng.dma_start(xp[g][:], xP[g][:])
            wq_sb = mp.tile([128, MT * 512], bf16, tag="wq_sb")
            nc.gpsimd.dma_start(wq_sb[:], Wq[:])
            vbb_sb = mp.tile([128, 512], bf16, tag="vbb")
            nc.gpsimd.dma_start(vbb_sb[:], vbb[:])
            qkb_sb = mp.tile([128, 8], f32, tag="qkb")
            nc.gpsimd.dma_start(qkb_sb[:], qkb[:])
            onz_sb = mp.tile([128, 8], bf16, tag="onz")
            nc.gpsimd.dma_start(onz_sb[:], onz[:])
            stair_sb = mp.tile([128, 256], bf16, tag="stair")
            nc.gpsimd.dma_start(stair_sb[:], stair2[:])
            one64_sb = mp.tile([1, 64], f32r, tag="one64")
            nc.gpsimd.dma_start(one64_sb[:], one64[:])
            wo_sb = mp.tile([128, 4 * DM], bf16, tag="wo_sb")
            nc.sync.dma_start(wo_sb[:], Wo[:])
            wk_sb = mp.tile([128, MT * 512], bf16, tag="wk_sb")
            nc.scalar.dma_start(wk_sb[:], Wk[:])

            def xsl(m, c0, c1):
                base = (m % 2) * S
                return xp[m // 2][:, base + c0:base + c1]

            # ---- persistent result tiles ----
            # v: [p, h*65+d] per 128-row k-tile; col 65h+64 = ones so the
            # merged S@V matmul (M=65) also produces the softmax denominator
            v_sb = [mp.tile([128, 520], bf16, tag=f"v{p}", name=f"v{p}")
                    for p in range(NKT)]
            qkT = {(t, pri): mp.tile([128, S], bf16, tag=f"{t}T{pri}",
                                     name=f"{t}T{pri}")
                   for t in ("q", "k") for pri in range(NPAIR)}
            at_sb = {(pri, i): mp.tile([128, 512], bf16, tag=f"at{pri}_{i}",
                                       name=f"at{pri}_{i}")
                     for pri in range(NPAIR) for i in range(NQP)}

            # ---- filler units (each ~0.9-1.9us of PE work) ----
            def vproj_evac(p, ps):
                vt = v_sb[p]
                nc.vector.tensor_add(
                    vt.rearrange("p (h c) -> p h c", c=65)[:, :, 0:64],
                    ps.rearrange("p (h c) -> p h c", c=64),
                    vbb_sb.rearrange("p (h c) -> p h c", c=64))
                nc.gpsimd.tensor_copy(
                    vt.rearrange("p (h c) -> p h c", c=65)[:, :, 64:65],
                    onz_sb.rearrange("p (h c) -> p h c", c=1))

            def unit_vproj(p):
                def emit():
                    ps = pp.tile([128, 512], f32, tag="proj", bufs=2)
                    for m in range(MT):
                        nc.tensor.matmul(
                            ps[:], xsl(m, p * 128, (p + 1) * 128),
                            wv_sb[:, m * 512:(m + 1) * 512],
                            start=(m == 0), stop=(m == MT - 1))
                    vproj_evac(p, ps)
                return emit

            def unit_qkproj(ti, pri, pb):
                def emit():
                    W = wq_sb if ti == 0 else wk_sb
                    out = qkT[("q" if ti == 0 else "k", pri)]
                    ps = pp.tile([128, 512], f32, tag="proj", bufs=2)
                    for m in range(MT):
                        nc.tensor.matmul(
                            ps[:],
                            W[:, m * 512 + pri * 128:m * 512 + (pri + 1) * 128],
                            xsl(m, pb * 512, (pb + 1) * 512),
                            start=(m == 0), stop=(m == MT - 1))
                    nc.vector.tensor_scalar_add(
                        out[:, pb * 512:(pb + 1) * 512], ps[:],
                        qkb_sb[:, 4 * ti + pri:4 * ti + pri + 1])
                return emit

            def unit_oproj(i, pt, dms=(0, 1)):
                def emit():
                    P = 4 * i + pt
                    yt = sp.tile([128, 512 * len(dms)], bf16, tag="yt",
                                 bufs=4, name="yt")
                    for n, dm in enumerate(dms):
                        ps = pp.tile([128, 512], f32, tag="proj", bufs=2)
                        for pri in range(NPAIR):
                            nc.tensor.matmul(
                                ps[:],
                                at_sb[(pri, i)][:, pt * 128:(pt + 1) * 128],
                                wo_sb[:, pri * DM + dm * 512:
                                      pri * DM + (dm + 1) * 512],
                                start=(pri == 0), stop=(pri == NPAIR - 1))
                        nc.vector.tensor_copy(
                            yt[:, n * 512:(n + 1) * 512], ps[:])
                    nc.sync.dma_start(
                        y[P * 128:(P + 1) * 128,
                          dms[0] * 512:(dms[-1] + 1) * 512], yt[:])
                return emit

            from collections import deque
            fillers = deque()   # (key, emit_fn, req); keys track emission
            chainq = deque()    # deferred normalization-chain ops
            emitted = set()
            chain_emitted = [0]

            def pop_chain():
                chainq.popleft()()
                chain_emitted[0] += 1

            def pop_filler():
                key, fn, req = fillers[0]
                if key in emitted:
                    fillers.popleft()
                    return
                # a filler may read tiles written by deferred chain ops;
                # force-emit the chain up to its snapshot first
                while chain_emitted[0] < req and chainq:
                    pop_chain()
                fillers.popleft()
                fn()
                emitted.add(key)
                units_left[0] -= 1

            def drain_until(keys):
                # engines run their queues in emission order, so a unit
                # producing data for block (pri, i) must be EMITTED before
                # the block's first consumer instruction
                while fillers and not keys <= emitted:
                    pop_filler()

            # preamble compute: v tiles 0..3 + pair-0 q/k block 0.
            # v0..3 are emitted as m-halves over four simultaneous psum
            # groups (2 proj bufs + 2 borrowed ev bufs) so their m0..3
            # matmuls run as soon as the first two x blobs land, ~10us
            # before the second pair arrives.
            pre_t = [pp.tile([128, 512], f32, tag="proj", bufs=2,
                             name=f"pre{p}") for p in range(2)]
            pre_ev = [pp.tile([128, 1024], f32, tag="ev", bufs=2,
                              name=f"prev{p}") for p in range(2)]
            pre_ps = [pre_t[0][:], pre_t[1][:],
                      pre_ev[0][:, 0:512], pre_ev[1][:, 0:512]]
            for p in range(4):
                for m in range(4):
                    nc.tensor.matmul(
                        pre_ps[p], xsl(m, p * 128, (p + 1) * 128),
                        wv_sb[:, m * 512:(m + 1) * 512],
                        start=(m == 0), stop=False)
            for p in range(4):
                for m in range(4, MT):
                    nc.tensor.matmul(
                        pre_ps[p], xsl(m, p * 128, (p + 1) * 128),
                        wv_sb[:, m * 512:(m + 1) * 512],
                        start=False, stop=(m == MT - 1))
                vproj_evac(p, pre_ps[p])
                emitted.add(("v", p))
            unit_qkproj(0, 0, 0)()
            unit_qkproj(1, 0, 0)()
            emitted.update({("q", 0, 0), ("k", 0, 0)})
            # q/k block 0 for the other pairs (i=0 needs them), then the
            # tiles later i-phases consume, interleaved in dependency order
            for pri in range(1, NPAIR):
                fillers.append((("q", pri, 0), unit_qkproj(0, pri, 0), 0))
                fillers.append((("k", pri, 0), unit_qkproj(1, pri, 0), 0))
            fillers.extend((("v", p), unit_vproj(p), 0) for p in (4, 5))
            for pri in range(NPAIR):
                fillers.append((("q", pri, 1), unit_qkproj(0, pri, 1), 0))
                fillers.append((("k", pri, 1), unit_qkproj(1, pri, 1), 0))
            fillers.extend((("v", p), unit_vproj(p), 0) for p in (6, 7))

            total_j = NPAIR * sum(4 * (i2 + 1) for i2 in range(NQP))
            done_j = [0]
            # total filler units over the whole schedule: 16 vproj + 32
            # qkproj + 16 oproj, minus the 6 emitted in the preamble
            units_left = [16 + 32 + 16 - 6]

            def block_keys(pri, i):
                return ({("q", pri, i)}
                        | {("k", pri, pb) for pb in range(i + 1)}
                        | {("v", p) for p in range(4 * (i + 1))})

            def mk_logits(pri, i):
                qT = qkT[("q", pri)]
                kT = qkT[("k", pri)]

                def emit_logits(j):
                    o = (j - 4 * i) * 128 if j >= 4 * i else 0
                    ev = pp.tile([128, 1024], f32, tag="ev", bufs=2,
                                 name="ev")
                    for h in range(2):
                        nc.tensor.matmul(
                            ev[:, h * 512 + o:(h + 1) * 512],
                            kT[64 * h:64 * h + 64, j * 128:(j + 1) * 128],
                            qT[64 * h:64 * h + 64,
                               i * 512 + o:(i + 1) * 512],
                            start=True, stop=True)
                    return ev, o
                return emit_logits

            pending_ev = {}   # (pri, i) -> {j: (ev, o)} primed cross-block

            def emit_block(pri, i, nxt=None):
                kmax = 4 * (i + 1)
                rem_j = total_j - done_j[0]
                stride = max(1, rem_j // max(1, units_left[0]))
                drain_until(block_keys(pri, i))
                for _ in range(2 if len(fillers) > 5 else 1):
                    if fillers:
                        pop_filler()
                ad = pp.tile([65, 1024], f32, tag="ad", bufs=1)
                emit_logits = mk_logits(pri, i)

                # logits run two j's ahead so the ACT exp stream never
                # starves across interleaved filler matmuls; the first
                # one or two may have been primed by the previous block
                evq = pending_ev.pop((pri, i), {})
                if 0 not in evq:
                    evq[0] = emit_logits(0)
                if kmax > 1 and 1 not in evq:
                    evq[1] = emit_logits(1)
                nxt_logits = mk_logits(*nxt) if nxt is not None else None
                for j in range(kmax):
                    ev, o = evq.pop(j)
                    sc = sp.tile([128, 1024], bf16, tag="sc", bufs=8)
                    if o:
                        nc.scalar.activation(
                            sc.rearrange("p (h c) -> p h c",
                                         c=512)[:, :, o:],
                            ev.rearrange("p (h c) -> p h c",
                                         c=512)[:, :, o:],
                            Exp, scale=0.125)
                    else:
                        nc.scalar.activation(sc[:], ev[:], Exp,
                                             scale=0.125)
                    if j >= 4 * i:
                        # staircase mask on the 128-wide diagonal strip
                        # (both heads in one DVE multiply)
                        strip = sc.rearrange(
                            "p (h c) -> p h c", c=512)[:, :, o:o + 128]
                        nc.vector.tensor_mul(
                            strip, strip,
                            stair_sb.rearrange("p (h c) -> p h c", c=128))
                    st = (j == 0)
                    sp_ = (j == kmax - 1)
                    vt = v_sb[j]
                    for h in range(2):
                        lh = 2 * pri + h
                        nc.tensor.matmul(
                            ad[0:65, h * 512 + o:(h + 1) * 512],
                            vt[:, lh * 65:lh * 65 + 65],
                            sc[:, h * 512 + o:(h + 1) * 512],
                            start=st, stop=sp_, skip_group_check=True)
                    if j + 2 < kmax:
                        evq[j + 2] = emit_logits(j + 2)
                    elif nxt is not None and j + 2 - kmax <= 1:
                        # prime the next block's first logits in the ev
                        # slots this block no longer needs, so its exp's
                        # are already done when its j-loop starts
                        jn = j + 2 - kmax
                        if jn == 0:
                            drain_until(block_keys(*nxt))
                        pending_ev.setdefault(nxt, {})[jn] = nxt_logits(jn)
                    if chainq:
                        pop_chain()
                    if fillers and (j % stride == stride - 1):
                        pop_filler()
                done_j[0] += kmax
                # normalization: evacuate ad via DVE (releases the PSUM
                # bank fast); the rest of the chain (DVE reciprocal ->
                # gpsimd partition broadcast -> two DVE multiplies -> DMA
                # repack) is deferred into the next block's j-loop so it
                # never head-of-line-blocks its engine queue.
                adc = sp.tile([65, 1024], f32, tag="adc", bufs=4,
                              name="adc")
                nc.vector.tensor_copy(adc[:], ad[:, :])
                at = at_sb[(pri, i)]
                tmp = sp.tile([64, 512], bf16, tag="tmp", bufs=4)
                lnt = sp.tile([1, 1024], f32, tag="lnt", bufs=2,
                              name="lnt")
                rec = sp.tile([1, 1024], f32r, tag="rec", bufs=2,
                              name="rec")

                den_src = ad if nxt is None else adc

                def chain_ops(adc=adc, at=at, tmp=tmp, rec=rec, lnt=lnt,
                              den_src=den_src):
                    # 1/den as exp(-ln(den)) on ACT (same table set as
                    # the softmax exp), then a PE broadcast matmul
                    # (one64 x rec) replicates it across 64 partitions
                    bch = [pp.tile([128, 512], f32, tag="proj", bufs=2,
                                   name=f"bc{hh}") for hh in range(2)]

                    def bc_mm(hh):
                        return lambda: nc.tensor.matmul(
                            bch[hh][0:64, :], one64_sb[:],
                            rec[:, hh * 512:(hh + 1) * 512],
                            start=True, stop=True)
                    yield lambda: nc.scalar.activation(
                        lnt[:], den_src[64:65, :], Ln)
                    yield lambda: nc.scalar.activation(
                        rec[:], lnt[:], Exp, scale=-1.0)
                    yield bc_mm(1)
                    yield bc_mm(0)
                    yield lambda: nc.vector.tensor_mul(
                        tmp[:], adc[0:64, 512:1024], bch[1][0:64, :])
                    yield lambda: nc.sync.dma_start(at[64:128, :],
                                                    tmp[:])
                    yield lambda: nc.vector.tensor_mul(
                        at[0:64, :], adc[0:64, 0:512], bch[0][0:64, :])

                if nxt is None:
                    # last block: no later j-loop will pop these; run the
                    # chain immediately so the final O units aren't stuck
                    # behind a fully serialized Ln/Exp/bc/mul/DMA chain
                    for op in chain_ops():
                        op()
                        chain_emitted[0] += 1
                else:
                    chainq.extend(chain_ops())

            # ---- attention: i-outer, pair-inner ----
            seq = [(pri, i) for i in range(NQP) for pri in range(NPAIR)]
            nxt_of = {seq[n]: seq[n + 1] for n in range(len(seq) - 1)}
            for i in range(NQP):
                for pri in range(NPAIR):
                    emit_block(pri, i, nxt_of.get((pri, i)))
                # O-projection units for block i become fillers for the
                # next phase (gated on all four pairs' chains finishing)
                req = chain_emitted[0] + len(chainq)
                if i == NQP - 1:
                    # split per dm-half at the end: finer tail overlap
                    for pt in range(4):
                        for dm in range(2):
                            fillers.append((("o", i, pt, dm),
                                            unit_oproj(i, pt, (dm,)), req))
                else:
                    for pt in range(4):
                        fillers.append((("o", i, pt), unit_oproj(i, pt),
                                        req))
                # stage the tiles phase i+2 consumes behind the O units
                # (the preamble already queued everything phases 0-1 need)
                if i + 2 < NQP:
                    queued = emitted | {f[0] for f in fillers}
                    for pri in range(NPAIR):
                        for key, ti in ((("q", pri, i + 2), 0),
                                        (("k", pri, i + 2), 1)):
                            if key not in queued:
                                fillers.append(
                                    (key, unit_qkproj(ti, pri, i + 2), 0))
                    fillers.extend(
                        (("v", p), unit_vproj(p), 0)
                        for p in range(4 * (i + 2), min(4 * (i + 3), NKT))
                        if ("v", p) not in queued)

            while chainq:
                pop_chain()
            while fillers:
                pop_filler()

    _split_multi_waits(nc, mybir)
    _cache["nc"] = nc
    return nc


def _host_inputs(x, Q_w, Q_b, K_w, K_b, V_w, V_b, O_w):
    import ml_dtypes
    bf = ml_dtypes.bfloat16
    stair = (np.arange(128)[:, None] <= np.arange(128)[None, :]).astype(bf)
    stair2 = np.concatenate([stair, stair], axis=1)
    in_maps = []
    for c in range(8):
        b, hs = c // 2, HPC * (c % 2)
        he = hs + HPC
        qb = Q_b[hs:he].reshape(512).astype(np.float32)
        kb = K_b[hs:he].reshape(512).astype(np.float32)
        qkb = np.zeros((128, 8), np.float32)
        for pri in range(NPAIR):
            qkb[:, pri] = qb[pri * 128:(pri + 1) * 128]
            qkb[:, 4 + pri] = kb[pri * 128:(pri + 1) * 128]
        xTb = np.ascontiguousarray(x[b].T).astype(bf)       # [DM, S]
        # weight blobs: [128, MT*512] with column block m = m-tile
        def wblob(W):  # W: [H/2==8 heads? no: [heads, DM, DH]] slice
            w2d = W[hs:he].transpose(1, 0, 2).reshape(DM, 512).astype(bf)
            return np.ascontiguousarray(
                w2d.reshape(MT, 128, 512).transpose(1, 0, 2).reshape(
                    128, MT * 512))
        wo2d = O_w[hs:he].reshape(512, DM).astype(bf)
        in_maps.append({
            **{f"xP{g}": np.ascontiguousarray(
                np.concatenate([xTb[2 * g * 128:(2 * g + 1) * 128, :],
                                xTb[(2 * g + 1) * 128:(2 * g + 2) * 128, :]],
                               axis=1)) for g in range(MT // 2)},
            "Wq": wblob(Q_w),
            "Wk": wblob(K_w),
            "Wv": wblob(V_w),
            "Wo": np.ascontiguousarray(
                wo2d.reshape(4, 128, DM).transpose(1, 0, 2).reshape(
                    128, 4 * DM)),
            "qkb": qkb,
            "vbb": np.tile(V_b[hs:he].reshape(1, 512), (128, 1)).astype(bf),
            "stair2": stair2,
            "onz": np.ones((128, 8), bf),
            "one64": np.ones((1, 64), np.float32),
        })
    return in_maps


def kernel(x, Q_w, Q_b, K_w, K_b, V_w, V_b, O_w, O_b, _trace=False):
    x = np.asarray(x, np.float32)
    args = [np.asarray(a, np.float32)
            for a in (Q_w, Q_b, K_w, K_b, V_w, V_b, O_w)]
    O_b = np.asarray(O_b, np.float32)

    nc = _build()
    from concourse.bass_utils import run_bass_kernel_spmd

    in_maps = _host_inputs(x, *args)
    res = run_bass_kernel_spmd(nc, in_maps, core_ids=list(range(8)),
                               trace=_trace)
    _cache["last_result"] = res
    out = np.empty((B, S, DM), np.float32)
    for b in range(B):
        out[b] = (res.results[2 * b]["y"].astype(np.float32)
                  + res.results[2 * b + 1]["y"].astype(np.float32) + O_b)
    return out


if __name__ == "__main__":
    # quick self-run with random inputs
    rng = np.random.default_rng(0)
    x = rng.standard_normal((B, S, DM), dtype=np.float32)
    shp = dict(Q_w=(H, DM, DH), Q_b=(H, DH), K_w=(H, DM, DH), K_b=(H, DH),
               V_w=(H, DM, DH), V_b=(H, DH), O_w=(H, DH, DM), O_b=(DM,))
    ins = {k: rng.standard_normal(v, dtype=np.float32) * 0.05
           for k, v in shp.items()}
    out = kernel(x, **ins)
    print("ran", out.shape, out.dtype)


# revision 47
# speedup vs baseline: 1.0017x; 1.0017x over previous
"""Multi-head causal attention (B=4, S=2048, H=16, Dh=64, Dm=1024) on 8
Trainium2 NeuronCores.

Sharding: core c handles batch b = c//2 and heads [8*(c%2), 8*(c%2)+8).
Each core computes its 8 heads' full attention + O-projection partial sum;
the host adds the two half-head partials per batch plus O_b.

v3 layout (all matmul inputs bf16, PSUM f32):
  - i-outer / pair-inner block order: for each q-block i, the four head
    pairs run back to back, then the O-projection units for block i
    become fillers for the i+1 phase.  This spreads the O-projection PE
    work and the y DMA across the whole timeline (v2 backloaded both).
  - Loads are single blob descriptors (per-descriptor cost ~2.6us for
    128 strided rows): wv halves lead the two HWDGE rings, then the x
    pair-blobs, then wo/wk; the slow SWDGE ring carries wq + constants.
    v0..3 run as m-halves over four concurrent PSUM groups (2 proj + 2
    borrowed ev bufs) so their first matmuls start when the first x
    blob pair lands.
  - Cross-block priming: each block emits the next block's first two
    logits tiles in its own tail, so the next block's exp stream is
    already running at the boundary.
  - Softmax: exp (ACT) is the only steady-state Scalar work besides the
    per-block Ln/Exp reciprocal (same ACT table set, no reloads).
    Denominators come from the ones-column of the v tiles (M=65 S@V);
    the reciprocal row is replicated across 64 partitions with a PE
    broadcast matmul (one64 x rec) and applied with two DVE multiplies;
    head B is repacked to partitions 64:127 with one SBUF->SBUF DMA.
  - O-projection PSUM is evacuated to bf16 yt tiles on DVE and DMA'd to
    y (bf16, halves the output traffic); the host sums the two per-batch
    partials in f32.  The last phase emits per-dm half units for a
    leaner drain, and the last block's chain runs undeferred.
  - Causal narrowing: for diagonal k-tiles only columns >= o are computed
    (logits matmul, exp, S@V); the 128-wide staircase strip is masked with
    one DVE multiply.
"""

import os
import sys

sys.path.insert(0, "/opt/trn_rl_repo")

import numpy as np

B, S, DM, H, DH = 4, 2048, 1024, 16, 64
HPC = 8          # heads per core
NPAIR = HPC // 2
PB = 512         # q block width
NQP = S // PB    # 4 q blocks
MT = DM // 128   # 8 m-tiles
NKT = S // 128   # 16 k tiles

_cache = {}


def _split_multi_waits(nc, mybir):
    # This container's walrus rejects >1 sync wait per instruction
    # ("Too many sync wait commands").  Move extra waits onto same-engine
    # NoOps right before the instruction; per-engine program order makes
    # this equivalent.
    ctr = 0
    for fn in nc.m.functions:
        for blk in fn.blocks:
            insts = list(blk.instructions)
            new_insts = []
            changed = False
            for inst in insts:
                si = getattr(inst, "sync_info", None)
                waits = list(si.on_wait) if (si is not None and si.on_wait) else []
                if len(waits) > 1:
                    changed = True
                    for w in waits[:-1]:
                        ctr += 1
                        new_insts.append(
                            mybir.InstNoOp(
                                name=f"waitsplit-{ctr}",
                                engine=inst.engine,
                                ins=[],
                                outs=[],
                                sync_info=mybir.SyncInfo(on_wait=[w], on_update=[]),
                            )
                        )
                    si.on_wait = [waits[-1]]
                new_insts.append(inst)
            if changed:
                blk.instructions = new_insts


def _patch_tile_drain(tile_mod, bass_mod):
    # Same walrus limitation hits the Tile kernel-tail drain (one wait per
    # ticked proc).  Chain the waits through single-wait sync NoOps.
    from concourse.vector_clock import ScopedClock, VectorClock

    def _drain_and_barrier(self, tick_clock, wait_clock):
        gc = tick_clock.global_clock
        n = len(gc)
        ticks = [gc[i] for i in range(n)]
        for p in [i for i in range(n) if ticks[i] > 0]:
            nop = self.nc.sync.nop(nofuse=True, hint="drain_wait_split")
            vc = VectorClock([ticks[j] if j == p else 0 for j in range(n)])
            wait_clock.add_sem_waits(nop.ins, ScopedClock({None: vc}))
        self.nc.sync.drain()
        self.nc.all_engine_barrier()
        assert self.sems is not None
        popped = self.nc._tile_sem_poison_stack.pop()
        assert popped is self._sem_poison
        self.nc.clear_and_free_semaphores(list(self.sems.allocated().values()))
        self.nc.all_engine_barrier()

    tile_mod.TileContext._drain_and_barrier = _drain_and_barrier


def _build():
    if "nc" in _cache:
        return _cache["nc"]

    import concourse.bass as bass
    import concourse.mybir as mybir
    import concourse.tile as tile
    from concourse import library_config

    _patch_tile_drain(tile, bass)

    f32 = mybir.dt.float32
    f32r = mybir.dt.float32r
    bf16 = mybir.dt.bfloat16
    Exp = mybir.ActivationFunctionType.Exp
    Ln = mybir.ActivationFunctionType.Ln

    nc = bass.Bass()
    xP = [nc.dram_tensor(f"xP{g}", [128, 2 * S], bf16, kind="ExternalInput")
          for g in range(MT // 2)]
    Wq = nc.dram_tensor("Wq", [128, MT * 512], bf16, kind="ExternalInput")
    Wk = nc.dram_tensor("Wk", [128, MT * 512], bf16, kind="ExternalInput")
    Wv = nc.dram_tensor("Wv", [128, MT * 512], bf16, kind="ExternalInput")
    Wo = nc.dram_tensor("Wo", [128, 4 * DM], bf16, kind="ExternalInput")
    qkb = nc.dram_tensor("qkb", [128, 8], f32, kind="ExternalInput")
    vbb = nc.dram_tensor("vbb", [128, 512], bf16, kind="ExternalInput")
    stair2 = nc.dram_tensor("stair2", [128, 256], bf16, kind="ExternalInput")
    onz = nc.dram_tensor("onz", [128, 8], bf16, kind="ExternalInput")
    one64 = nc.dram_tensor("one64", [1, 64], f32r, kind="ExternalInput")
    y = nc.dram_tensor("y", [S, DM], bf16, kind="ExternalOutput")

    with tile.TileContext(nc) as tc:
        with nc.allow_low_precision(reason="bf16 tiles feeding the PE"), \
             tc.tile_pool(name="mp", bufs=1) as mp, \
             tc.tile_pool(name="sp", bufs=1) as sp, \
             tc.tile_pool(name="pp", bufs=1, space="PSUM") as pp:

            # ---- input loads ----
            # Each load is ONE blob descriptor (per-descriptor cost is
            # ~2-3us for 128 strided partition rows, so count matters).
            # The HWDGE rings (sync/scalar, ~115GB/s) carry wv halves
            # first, then the x pair-blobs, then wo/wk; the slow gpsimd
            # SWDGE ring (~45GB/s) carries only wq + small constants.
            wv_sb = mp.tile([128, MT * 512], bf16, tag="wv_sb")
            nc.sync.dma_start(wv_sb[:, 0:MT * 256], Wv[:, 0:MT * 256])
            nc.scalar.dma_start(wv_sb[:, MT * 256:], Wv[:, MT * 256:])
            xp = [mp.tile([128, 2 * S], bf16, tag=f"xp{g}", name=f"xp{g}")
                  for g in range(MT // 2)]
            for g in range(MT // 2):
                eng = nc.sync if g % 2 == 0 else nc.scalar
                eng.dma_start(xp[g][:], xP[g][:])
            wq_sb = mp.tile([128, MT * 512], bf16, tag="wq_sb")
            nc.gpsimd.dma_start(wq_sb[:], Wq[:])
            vbb_sb = mp.tile([128, 512], bf16, tag="vbb")
            nc.gpsimd.dma_start(vbb_sb[:], vbb[:])
            qkb_sb = mp.tile([128, 8], f32, tag="qkb")
            nc.gpsimd.dma_start(qkb_sb[:], qkb[:])
            onz_sb = mp.tile([128, 8], bf16, tag="onz")
            nc.gpsimd.dma_start(onz_sb[:], onz[:])
            stair_sb = mp.tile([128, 256], bf16, tag="stair")
            nc.gpsimd.dma_start(stair_sb[:], stair2[:])
            one64_sb = mp.tile([1, 64], f32r, tag="one64")
            nc.gpsimd.dma_start(one64_sb[:], one64[:])
            wo_sb = mp.tile([128, 4 * DM], bf16, tag="wo_sb")
            nc.sync.dma_start(wo_sb[:], Wo[:])
            wk_sb = mp.tile([128, MT * 512], bf16, tag="wk_sb")
            nc.scalar.dma_start(wk_sb[:], Wk[:])

            def xsl(m, c0, c1):
                base = (m % 2) * S
                return xp[m // 2][:, base + c0:base + c1]

            # ---- persistent result tiles ----
            # v: [p, h*65+d] per 128-row k-tile; col 65h+64 = ones so the
            # merged S@V matmul (M=65) also produces the softmax denominator
            v_sb = [mp.tile([128, 520], bf16, tag=f"v{p}", name=f"v{p}")
                    for p in range(NKT)]
            qkT = {(t, pri): mp.tile([128, S], bf16, tag=f"{t}T{pri}",
                                     name=f"{t}T{pri}")
                   for t in ("q", "k") for pri in range(NPAIR)}
            at_sb = {(pri, i): mp.tile([128, 512], bf16, tag=f"at{pri}_{i}",
                                       name=f"at{pri}_{i}")
                     for pri in range(NPAIR) for i in range(NQP)}

            # ---- filler units (each ~0.9-1.9us of PE work) ----
            def vproj_evac(p, ps):
                vt = v_sb[p]
                nc.vector.tensor_add(
                    vt.rearrange("p (h c) -> p h c", c=65)[:, :, 0:64],
                    ps.rearrange("p (h c) -> p h c", c=64),
                    vbb_sb.rearrange("p (h c) -> p h c", c=64))
                nc.gpsimd.tensor_copy(
                    vt.rearrange("p (h c) -> p h c", c=65)[:, :, 64:65],
                    onz_sb.rearrange("p (h c) -> p h c", c=1))

            def unit_vproj(p):
                def emit():
                    ps = pp.tile([128, 512], f32, tag="proj", bufs=2)
                    for m in range(MT):
                        nc.tensor.matmul(
                            ps[:], xsl(m, p * 128, (p + 1) * 128),
                            wv_sb[:, m * 512:(m + 1) * 512],
                            start=(m == 0), stop=(m == MT - 1))
                    vproj_evac(p, ps)
                return emit

            def unit_qkproj(ti, pri, pb):
                def emit():
                    W = wq_sb if ti == 0 else wk_sb
                    out = qkT[("q" if ti == 0 else "k", pri)]
                    ps = pp.tile([128, 512], f32, tag="proj", bufs=2)
                    for m in range(MT):
                        nc.tensor.matmul(
                            ps[:],
                            W[:, m * 512 + pri * 128:m * 512 + (pri + 1) * 128],
                            xsl(m, pb * 512, (pb + 1) * 512),
                            start=(m == 0), stop=(m == MT - 1))
                    nc.vector.tensor_scalar_add(
                        out[:, pb * 512:(pb + 1) * 512], ps[:],
                        qkb_sb[:, 4 * ti + pri:4 * ti + pri + 1])
                return emit

            def unit_oproj(i, pt, dms=(0, 1)):
                def emit():
                    P = 4 * i + pt
                    yt = sp.tile([128, 512 * len(dms)], bf16, tag="yt",
                                 bufs=4, name="yt")
                    for n, dm in enumerate(dms):
                        ps = pp.tile([128, 512], f32, tag="proj", bufs=2)
                        for pri in range(NPAIR):
                            nc.tensor.matmul(
                                ps[:],
                                at_sb[(pri, i)][:, pt * 128:(pt + 1) * 128],
                                wo_sb[:, pri * DM + dm * 512:
                                      pri * DM + (dm + 1) * 512],
                                start=(pri == 0), stop=(pri == NPAIR - 1))
                        nc.vector.tensor_copy(
                            yt[:, n * 512:(n + 1) * 512], ps[:])
                    nc.sync.dma_start(
                        y[P * 128:(P + 1) * 128,
                          dms[0] * 512:(dms[-1] + 1) * 512], yt[:])
                return emit

            from collections import deque
            fillers = deque()   # (key, emit_fn, req); keys track emission
            chainq = deque()    # deferred normalization-chain ops
            emitted = set()
            chain_emitted = [0]

            def pop_chain():
                chainq.popleft()()
                chain_emitted[0] += 1

            def pop_filler():
                key, fn, req = fillers[0]
                if key in emitted:
                    fillers.popleft()
                    return
                # a filler may read tiles written by deferred chain ops;
                # force-emit the chain up to its snapshot first
                while chain_emitted[0] < req and chainq:
                    pop_chain()
                fillers.popleft()
                fn()
                emitted.add(key)
                units_left[0] -= 1

            def drain_until(keys):
                # engines run their queues in emission order, so a unit
                # producing data for block (pri, i) must be EMITTED before
                # the block's first consumer instruction
                while fillers and not keys <= emitted:
                    pop_filler()

            # preamble compute: v tiles 0..3 + pair-0 q/k block 0.
            # v0..3 are emitted as m-halves over four simultaneous psum
            # groups (2 proj bufs + 2 borrowed ev bufs) so their m0..3
            # matmuls run as soon as the first two x blobs land, ~10us
            # before the second pair arrives.
            pre_t = [pp.tile([128, 512], f32, tag="proj", bufs=2,
                             name=f"pre{p}") for p in range(2)]
            pre_ev = [pp.tile([128, 1024], f32, tag="ev", bufs=2,
                              name=f"prev{p}") for p in range(2)]
            pre_ps = [pre_t[0][:], pre_t[1][:],
                      pre_ev[0][:, 0:512], pre_ev[1][:, 0:512]]
            for p in range(4):
                for m in range(4):
                    nc.tensor.matmul(
                        pre_ps[p], xsl(m, p * 128, (p + 1) * 128),
                        wv_sb[:, m * 512:(m + 1) * 512],
                        start=(m == 0), stop=False)
            for p in range(4):
                for m in range(4, MT):
                    nc.tensor.matmul(
                        pre_ps[p], xsl(m, p * 128, (p + 1) * 128),
                        wv_sb[:, m * 512:(m + 1) * 512],
                        start=False, stop=(m == MT - 1))
                vproj_evac(p, pre_ps[p])
                emitted.add(("v", p))
            unit_qkproj(0, 0, 0)()
            unit_qkproj(1, 0, 0)()
            emitted.update({("q", 0, 0), ("k", 0, 0)})
            # q/k block 0 for the other pairs (i=0 needs them), then the
            # tiles later i-phases consume, interleaved in dependency order
            for pri in range(1, NPAIR):
                fillers.append((("q", pri, 0), unit_qkproj(0, pri, 0), 0))
                fillers.append((("k", pri, 0), unit_qkproj(1, pri, 0), 0))
            fillers.extend((("v", p), unit_vproj(p), 0) for p in (4, 5))
            for pri in range(NPAIR):
                fillers.append((("q", pri, 1), unit_qkproj(0, pri, 1), 0))
                fillers.append((("k", pri, 1), unit_qkproj(1, pri, 1), 0))
            fillers.extend((("v", p), unit_vproj(p), 0) for p in (6, 7))

            total_j = NPAIR * sum(4 * (i2 + 1) for i2 in range(NQP))
            done_j = [0]
            # total filler units over the whole schedule: 16 vproj + 32
            # qkproj + 16 oproj, minus the 6 emitted in the preamble
            units_left = [16 + 32 + 16 - 6]

            def block_keys(pri, i):
                return ({("q", pri, i)}
                        | {("k", pri, pb) for pb in range(i + 1)}
                        | {("v", p) for p in range(4 * (i + 1))})

            def mk_logits(pri, i):
                qT = qkT[("q", pri)]
                kT = qkT[("k", pri)]

                def emit_logits(j):
                    o = (j - 4 * i) * 128 if j >= 4 * i else 0
                    ev = pp.tile([128, 1024], f32, tag="ev", bufs=2,
                                 name="ev")
                    for h in range(2):
                        nc.tensor.matmul(
                            ev[:, h * 512 + o:(h + 1) * 512],
                            kT[64 * h:64 * h + 64, j * 128:(j + 1) * 128],
                            qT[64 * h:64 * h + 64,
                               i * 512 + o:(i + 1) * 512],
                            start=True, stop=True)
                    return ev, o
                return emit_logits

            pending_ev = {}   # (pri, i) -> {j: (ev, o)} primed cross-block

            def emit_block(pri, i, nxt=None):
                kmax = 4 * (i + 1)
                rem_j = total_j - done_j[0]
                stride = max(1, rem_j // max(1, units_left[0]))
                drain_until(block_keys(pri, i))
                for _ in range(2 if len(fillers) > 5 else 1):
                    if fillers:
                        pop_filler()
                ad = pp.tile([65, 1024], f32, tag="ad", bufs=1)
                emit_logits = mk_logits(pri, i)

                # logits run two j's ahead so the ACT exp stream never
                # starves across interleaved filler matmuls; the first
                # one or two may have been primed by the previous block
                evq = pending_ev.pop((pri, i), {})
                if 0 not in evq:
                    evq[0] = emit_logits(0)
                if kmax > 1 and 1 not in evq:
                    evq[1] = emit_logits(1)
                nxt_logits = mk_logits(*nxt) if nxt is not None else None
                for j in range(kmax):
                    ev, o = evq.pop(j)
                    sc = sp.tile([128, 1024], bf16, tag="sc", bufs=8)
                    if o:
                        nc.scalar.activation(
                            sc.rearrange("p (h c) -> p h c",
                                         c=512)[:, :, o:],
                            ev.rearrange("p (h c) -> p h c",
                                         c=512)[:, :, o:],
                            Exp, scale=0.125)
                    else:
                        nc.scalar.activation(sc[:], ev[:], Exp,
                                             scale=0.125)
                    if j >= 4 * i:
                        # staircase mask on the 128-wide diagonal strip
                        # (both heads in one DVE multiply)
                        strip = sc.rearrange(
                            "p (h c) -> p h c", c=512)[:, :, o:o + 128]
                        nc.vector.tensor_mul(
                            strip, strip,
                            stair_sb.rearrange("p (h c) -> p h c", c=128))
                    st = (j == 0)
                    sp_ = (j == kmax - 1)
                    vt = v_sb[j]
                    for h in range(2):
                        lh = 2 * pri + h
                        nc.tensor.matmul(
                            ad[0:65, h * 512 + o:(h + 1) * 512],
                            vt[:, lh * 65:lh * 65 + 65],
                            sc[:, h * 512 + o:(h + 1) * 512],
                            start=st, stop=sp_, skip_group_check=True)
                    if j + 2 < kmax:
                        evq[j + 2] = emit_logits(j + 2)
                    elif nxt is not None and j + 2 - kmax <= 1:
                        # prime the next block's first logits in the ev
                        # slots this block no longer needs, so its exp's
                        # are already done when its j-loop starts
                        jn = j + 2 - kmax
                        if jn == 0:
                            drain_until(block_keys(*nxt))
                        pending_ev.setdefault(nxt, {})[jn] = nxt_logits(jn)
                    if chainq:
                        pop_chain()
                    if fillers and (j % stride == stride - 1):
                        pop_filler()
                done_j[0] += kmax
                # normalization: evacuate ad via DVE (releases the PSUM
                # bank fast); the rest of the chain (DVE reciprocal ->
                # gpsimd partition broadcast -> two DVE multiplies -> DMA
                # repack) is deferred into the next block's j-loop so it
                # never head-of-line-blocks its engine queue.
                adc = sp.tile([65, 1024], f32, tag="adc", bufs=4,
                              name="adc")
                nc.vector.tensor_copy(adc[:], ad[:, :])
                at = at_sb[(pri, i)]
                tmp = sp.tile([64, 512], bf16, tag="tmp", bufs=4)
                lnt = sp.tile([1, 1024], f32, tag="lnt", bufs=2,
                              name="lnt")
                rec = sp.tile([1, 1024], f32r, tag="rec", bufs=2,
                              name="rec")

                def chain_ops(adc=adc, at=at, tmp=tmp, rec=rec, lnt=lnt):
                    # 1/den as exp(-ln(den)) on ACT (same table set as
                    # the softmax exp), then a PE broadcast matmul
                    # (one64 x rec) replicates it across 64 partitions
                    bch = [pp.tile([128, 512], f32, tag="proj", bufs=2,
                                   name=f"bc{hh}") for hh in range(2)]

                    def bc_mm(hh):
                        return lambda: nc.tensor.matmul(
                            bch[hh][0:64, :], one64_sb[:],
                            rec[:, hh * 512:(hh + 1) * 512],
                            start=True, stop=True)
                    yield lambda: nc.scalar.activation(
                        lnt[:], adc[64:65, :], Ln)
                    yield lambda: nc.scalar.activation(
                        rec[:], lnt[:], Exp, scale=-1.0)
                    yield bc_mm(1)
                    yield bc_mm(0)
                    yield lambda: nc.vector.tensor_mul(
                        tmp[:], adc[0:64, 512:1024], bch[1][0:64, :])
                    yield lambda: nc.sync.dma_start(at[64:128, :],
                                                    tmp[:])
                    yield lambda: nc.vector.tensor_mul(
                        at[0:64, :], adc[0:64, 0:512], bch[0][0:64, :])

                if nxt is None:
                    # last block: no later j-loop will pop these; run the
                    # chain immediately so the final O units aren't stuck
                    # behind a fully serialized Ln/Exp/bc/mul/DMA chain
                    for op in chain_ops():
                        op()
                        chain_emitted[0] += 1
                else:
                    chainq.extend(chain_ops())

            # ---- attention: i-outer, pair-inner ----
            seq = [(pri, i) for i in range(NQP) for pri in range(NPAIR)]
            nxt_of = {seq[n]: seq[n + 1] for n in range(len(seq) - 1)}
            for i in range(NQP):
                for pri in range(NPAIR):
                    emit_block(pri, i, nxt_of.get((pri, i)))
                # O-projection units for block i become fillers for the
                # next phase (gated on all four pairs' chains finishing)
                req = chain_emitted[0] + len(chainq)
                if i == NQP - 1:
                    # split per dm-half at the end: finer tail overlap
                    for pt in range(4):
                        for dm in range(2):
                            fillers.append((("o", i, pt, dm),
                                            unit_oproj(i, pt, (dm,)), req))
                else:
                    for pt in range(4):
                        fillers.append((("o", i, pt), unit_oproj(i, pt),
                                        req))
                # stage the tiles phase i+2 consumes behind the O units
                # (the preamble already queued everything phases 0-1 need)
                if i + 2 < NQP:
                    queued = emitted | {f[0] for f in fillers}
                    for pri in range(NPAIR):
                        for key, ti in ((("q", pri, i + 2), 0),
                                        (("k", pri, i + 2), 1)):
                            if key not in queued:
                                fillers.append(
                                    (key, unit_qkproj(ti, pri, i + 2), 0))
                    fillers.extend(
                        (("v", p), unit_vproj(p), 0)
                        for p in range(4 * (i + 2), min(4 * (i + 3), NKT))
                        if ("v", p) not in queued)

            while chainq:
                pop_chain()
            while fillers:
                pop_filler()

    _split_multi_waits(nc, mybir)
    _cache["nc"] = nc
    return nc


def _host_inputs(x, Q_w, Q_b, K_w, K_b, V_w, V_b, O_w):
    import ml_dtypes
    bf = ml_dtypes.bfloat16
    stair = (np.arange(128)[:, None] <= np.arange(128)[None, :]).astype(bf)
    stair2 = np.concatenate([stair, stair], axis=1)
    in_maps = []
    for c in range(8):
        b, hs = c // 2, HPC * (c % 2)
        he = hs + HPC
        qb = Q_b[hs:he].reshape(512).astype(np.float32)
        kb = K_b[hs:he].reshape(512).astype(np.float32)
        qkb = np.zeros((128, 8), np.float32)
        for pri in range(NPAIR):
            qkb[:, pri] = qb[pri * 128:(pri + 1) * 128]
            qkb[:, 4 + pri] = kb[pri * 128:(pri + 1) * 128]
        xTb = np.ascontiguousarray(x[b].T).astype(bf)       # [DM, S]
        # weight blobs: [128, MT*512] with column block m = m-tile
        def wblob(W):  # W: [H/2==8 heads? no: [heads, DM, DH]] slice
            w2d = W[hs:he].transpose(1, 0, 2).reshape(DM, 512).astype(bf)
            return np.ascontiguousarray(
                w2d.reshape(MT, 128, 512).transpose(1, 0, 2).reshape(
                    128, MT * 512))
        wo2d = O_w[hs:he].reshape(512, DM).astype(bf)
        in_maps.append({
            **{f"xP{g}": np.ascontiguousarray(
                np.concatenate([xTb[2 * g * 128:(2 * g + 1) * 128, :],
                                xTb[(2 * g + 1) * 128:(2 * g + 2) * 128, :]],
                               axis=1)) for g in range(MT // 2)},
            "Wq": wblob(Q_w),
            "Wk": wblob(K_w),
            "Wv": wblob(V_w),
            "Wo": np.ascontiguousarray(
                wo2d.reshape(4, 128, DM).transpose(1, 0, 2).reshape(
                    128, 4 * DM)),
            "qkb": qkb,
            "vbb": np.tile(V_b[hs:he].reshape(1, 512), (128, 1)).astype(bf),
            "stair2": stair2,
            "onz": np.ones((128, 8), bf),
            "one64": np.ones((1, 64), np.float32),
        })
    return in_maps


def kernel(x, Q_w, Q_b, K_w, K_b, V_w, V_b, O_w, O_b, _trace=False):
    x = np.asarray(x, np.float32)
    args = [np.asarray(a, np.float32)
            for a in (Q_w, Q_b, K_w, K_b, V_w, V_b, O_w)]
    O_b = np.asarray(O_b, np.float32)

    nc = _build()
    from concourse.bass_utils import run_bass_kernel_spmd

    in_maps = _host_inputs(x, *args)
    res = run_bass_kernel_spmd(nc, in_maps, core_ids=list(range(8)),
                               trace=_trace)
    _cache["last_result"] = res
    out = np.empty((B, S, DM), np.float32)
    for b in range(B):
        out[b] = (res.results[2 * b]["y"].astype(np.float32)
                  + res.results[2 * b + 1]["y"].astype(np.float32) + O_b)
    return out


if __name__ == "__main__":
    # quick self-run with random inputs
    rng = np.random.default_rng(0)
    x = rng.standard_normal((B, S, DM), dtype=np.float32)
    shp = dict(Q_w=(H, DM, DH), Q_b=(H, DH), K_w=(H, DM, DH), K_b=(H, DH),
               V_w=(H, DM, DH), V_b=(H, DH), O_w=(H, DH, DM), O_b=(DM,))
    ins = {k: rng.standard_normal(v, dtype=np.float32) * 0.05
           for k, v in shp.items()}
    out = kernel(x, **ins)
    print("ran", out.shape, out.dtype)


# revision 49
# speedup vs baseline: 1.0093x; 1.0077x over previous
"""Multi-head causal attention (B=4, S=2048, H=16, Dh=64, Dm=1024) on 8
Trainium2 NeuronCores.

Sharding: core c handles batch b = c//2 and heads [8*(c%2), 8*(c%2)+8).
Each core computes its 8 heads' full attention + O-projection partial sum;
the host adds the two half-head partials per batch plus O_b.

v3 layout (all matmul inputs bf16, PSUM f32):
  - i-outer / pair-inner block order: for each q-block i, the four head
    pairs run back to back, then the O-projection units for block i
    become fillers for the i+1 phase.  This spreads the O-projection PE
    work and the y DMA across the whole timeline (v2 backloaded both).
  - Loads are single blob descriptors (per-descriptor cost ~2.6us for
    128 strided rows): wv halves lead the two HWDGE rings, then the x
    pair-blobs, then wo/wk; the slow SWDGE ring carries wq + constants.
    v0..3 run as m-halves over four concurrent PSUM groups (2 proj + 2
    borrowed ev bufs) so their first matmuls start when the first x
    blob pair lands.
  - Cross-block priming: each block emits the next block's first two
    logits tiles in its own tail, so the next block's exp stream is
    already running at the boundary.
  - Softmax: exp (ACT) is the only steady-state Scalar work besides the
    per-block Ln/Exp reciprocal (same ACT table set, no reloads).
    Denominators come from the ones-column of the v tiles (M=65 S@V);
    the reciprocal row is replicated across 64 partitions with a PE
    broadcast matmul (one64 x rec) and applied with two DVE multiplies;
    head B is repacked to partitions 64:127 with one SBUF->SBUF DMA.
  - O-projection PSUM is evacuated to bf16 yt tiles on DVE and DMA'd to
    y (bf16, halves the output traffic); the host sums the two per-batch
    partials in f32.  The last phase emits per-dm half units for a
    leaner drain, and the last block's chain runs undeferred.
  - Causal narrowing: for diagonal k-tiles only columns >= o are computed
    (logits matmul, exp, S@V); the 128-wide staircase strip is masked with
    one DVE multiply.
"""

import os
import sys

sys.path.insert(0, "/opt/trn_rl_repo")

import numpy as np

B, S, DM, H, DH = 4, 2048, 1024, 16, 64
HPC = 8          # heads per core
NPAIR = HPC // 2
PB = 512         # q block width
NQP = S // PB    # 4 q blocks
MT = DM // 128   # 8 m-tiles
NKT = S // 128   # 16 k tiles

_cache = {}


def _split_multi_waits(nc, mybir):
    # This container's walrus rejects >1 sync wait per instruction
    # ("Too many sync wait commands").  Move extra waits onto same-engine
    # NoOps right before the instruction; per-engine program order makes
    # this equivalent.
    ctr = 0
    for fn in nc.m.functions:
        for blk in fn.blocks:
            insts = list(blk.instructions)
            new_insts = []
            changed = False
            for inst in insts:
                si = getattr(inst, "sync_info", None)
                waits = list(si.on_wait) if (si is not None and si.on_wait) else []
                if len(waits) > 1:
                    changed = True
                    for w in waits[:-1]:
                        ctr += 1
                        new_insts.append(
                            mybir.InstNoOp(
                                name=f"waitsplit-{ctr}",
                                engine=inst.engine,
                                ins=[],
                                outs=[],
                                sync_info=mybir.SyncInfo(on_wait=[w], on_update=[]),
                            )
                        )
                    si.on_wait = [waits[-1]]
                new_insts.append(inst)
            if changed:
                blk.instructions = new_insts


def _patch_tile_drain(tile_mod, bass_mod):
    # Same walrus limitation hits the Tile kernel-tail drain (one wait per
    # ticked proc).  Chain the waits through single-wait sync NoOps.
    from concourse.vector_clock import ScopedClock, VectorClock

    def _drain_and_barrier(self, tick_clock, wait_clock):
        gc = tick_clock.global_clock
        n = len(gc)
        ticks = [gc[i] for i in range(n)]
        for p in [i for i in range(n) if ticks[i] > 0]:
            nop = self.nc.sync.nop(nofuse=True, hint="drain_wait_split")
            vc = VectorClock([ticks[j] if j == p else 0 for j in range(n)])
            wait_clock.add_sem_waits(nop.ins, ScopedClock({None: vc}))
        self.nc.sync.drain()
        self.nc.all_engine_barrier()
        assert self.sems is not None
        popped = self.nc._tile_sem_poison_stack.pop()
        assert popped is self._sem_poison
        self.nc.clear_and_free_semaphores(list(self.sems.allocated().values()))
        self.nc.all_engine_barrier()

    tile_mod.TileContext._drain_and_barrier = _drain_and_barrier


def _build():
    if "nc" in _cache:
        return _cache["nc"]

    import concourse.bass as bass
    import concourse.mybir as mybir
    import concourse.tile as tile
    from concourse import library_config

    _patch_tile_drain(tile, bass)

    f32 = mybir.dt.float32
    f32r = mybir.dt.float32r
    bf16 = mybir.dt.bfloat16
    Exp = mybir.ActivationFunctionType.Exp
    Ln = mybir.ActivationFunctionType.Ln

    nc = bass.Bass()
    xP = [nc.dram_tensor(f"xP{g}", [128, 2 * S], bf16, kind="ExternalInput")
          for g in range(MT // 2)]
    Wq = nc.dram_tensor("Wq", [128, MT * 512], bf16, kind="ExternalInput")
    Wk = nc.dram_tensor("Wk", [128, MT * 512], bf16, kind="ExternalInput")
    Wv = nc.dram_tensor("Wv", [128, MT * 512], bf16, kind="ExternalInput")
    Wo = nc.dram_tensor("Wo", [128, 4 * DM], bf16, kind="ExternalInput")
    qkb = nc.dram_tensor("qkb", [128, 8], f32, kind="ExternalInput")
    vbb = nc.dram_tensor("vbb", [128, 512], bf16, kind="ExternalInput")
    stair2 = nc.dram_tensor("stair2", [128, 256], bf16, kind="ExternalInput")
    onz = nc.dram_tensor("onz", [128, 8], bf16, kind="ExternalInput")
    one64 = nc.dram_tensor("one64", [1, 64], f32r, kind="ExternalInput")
    y = nc.dram_tensor("y", [S, DM], bf16, kind="ExternalOutput")

    with tile.TileContext(nc) as tc:
        with nc.allow_low_precision(reason="bf16 tiles feeding the PE"), \
             tc.tile_pool(name="mp", bufs=1) as mp, \
             tc.tile_pool(name="sp", bufs=1) as sp, \
             tc.tile_pool(name="pp", bufs=1, space="PSUM") as pp:

            # ---- input loads ----
            # Each load is ONE blob descriptor (per-descriptor cost is
            # ~2-3us for 128 strided partition rows, so count matters).
            # The HWDGE rings (sync/scalar, ~115GB/s) carry wv halves
            # first, then the x pair-blobs, then wo/wk; the slow gpsimd
            # SWDGE ring (~45GB/s) carries only wq + small constants.
            wv_sb = mp.tile([128, MT * 512], bf16, tag="wv_sb")
            nc.sync.dma_start(wv_sb[:, 0:MT * 256], Wv[:, 0:MT * 256])
            nc.scalar.dma_start(wv_sb[:, MT * 256:], Wv[:, MT * 256:])
            xp = [mp.tile([128, 2 * S], bf16, tag=f"xp{g}", name=f"xp{g}")
                  for g in range(MT // 2)]
            for g in range(MT // 2):
                eng = nc.sync if g % 2 == 0 else nc.scalar
                eng.dma_start(xp[g][:], xP[g][:])
            wq_sb = mp.tile([128, MT * 512], bf16, tag="wq_sb")
            nc.gpsimd.dma_start(wq_sb[:], Wq[:])
            vbb_sb = mp.tile([128, 512], bf16, tag="vbb")
            nc.gpsimd.dma_start(vbb_sb[:], vbb[:])
            qkb_sb = mp.tile([128, 8], f32, tag="qkb")
            nc.gpsimd.dma_start(qkb_sb[:], qkb[:])
            onz_sb = mp.tile([128, 8], bf16, tag="onz")
            nc.gpsimd.dma_start(onz_sb[:], onz[:])
            stair_sb = mp.tile([128, 256], bf16, tag="stair")
            nc.gpsimd.dma_start(stair_sb[:], stair2[:])
            one64_sb = mp.tile([1, 64], f32r, tag="one64")
            nc.gpsimd.dma_start(one64_sb[:], one64[:])
            wo_sb = mp.tile([128, 4 * DM], bf16, tag="wo_sb")
            nc.sync.dma_start(wo_sb[:], Wo[:])
            wk_sb = mp.tile([128, MT * 512], bf16, tag="wk_sb")
            nc.scalar.dma_start(wk_sb[:], Wk[:])

            def xsl(m, c0, c1):
                base = (m % 2) * S
                return xp[m // 2][:, base + c0:base + c1]

            # ---- persistent result tiles ----
            # v: [p, h*65+d] per 128-row k-tile; col 65h+64 = ones so the
            # merged S@V matmul (M=65) also produces the softmax denominator
            v_sb = [mp.tile([128, 520], bf16, tag=f"v{p}", name=f"v{p}")
                    for p in range(NKT)]
            qkT = {(t, pri): mp.tile([128, S], bf16, tag=f"{t}T{pri}",
                                     name=f"{t}T{pri}")
                   for t in ("q", "k") for pri in range(NPAIR)}
            at_sb = {(pri, i): mp.tile([128, 512], bf16, tag=f"at{pri}_{i}",
                                       name=f"at{pri}_{i}")
                     for pri in range(NPAIR) for i in range(NQP)}

            # ---- filler units (each ~0.9-1.9us of PE work) ----
            def vproj_evac(p, ps):
                vt = v_sb[p]
                nc.vector.tensor_add(
                    vt.rearrange("p (h c) -> p h c", c=65)[:, :, 0:64],
                    ps.rearrange("p (h c) -> p h c", c=64),
                    vbb_sb.rearrange("p (h c) -> p h c", c=64))
                nc.gpsimd.tensor_copy(
                    vt.rearrange("p (h c) -> p h c", c=65)[:, :, 64:65],
                    onz_sb.rearrange("p (h c) -> p h c", c=1))

            def unit_vproj(p):
                def emit():
                    ps = pp.tile([128, 512], f32, tag="proj", bufs=2)
                    for m in range(MT):
                        nc.tensor.matmul(
                            ps[:], xsl(m, p * 128, (p + 1) * 128),
                            wv_sb[:, m * 512:(m + 1) * 512],
                            start=(m == 0), stop=(m == MT - 1))
                    vproj_evac(p, ps)
                return emit

            def unit_qkproj(ti, pri, pb):
                def emit():
                    W = wq_sb if ti == 0 else wk_sb
                    out = qkT[("q" if ti == 0 else "k", pri)]
                    ps = pp.tile([128, 512], f32, tag="proj", bufs=2)
                    for m in range(MT):
                        nc.tensor.matmul(
                            ps[:],
                            W[:, m * 512 + pri * 128:m * 512 + (pri + 1) * 128],
                            xsl(m, pb * 512, (pb + 1) * 512),
                            start=(m == 0), stop=(m == MT - 1))
                    nc.vector.tensor_scalar_add(
                        out[:, pb * 512:(pb + 1) * 512], ps[:],
                        qkb_sb[:, 4 * ti + pri:4 * ti + pri + 1])
                return emit

            def unit_oproj(i, pt, dms=(0, 1)):
                def emit():
                    P = 4 * i + pt
                    yt = sp.tile([128, 512 * len(dms)], bf16, tag="yt",
                                 bufs=4, name="yt")
                    for n, dm in enumerate(dms):
                        ps = pp.tile([128, 512], f32, tag="proj", bufs=2)
                        for pri in range(NPAIR):
                            nc.tensor.matmul(
                                ps[:],
                                at_sb[(pri, i)][:, pt * 128:(pt + 1) * 128],
                                wo_sb[:, pri * DM + dm * 512:
                                      pri * DM + (dm + 1) * 512],
                                start=(pri == 0), stop=(pri == NPAIR - 1))
                        nc.vector.tensor_copy(
                            yt[:, n * 512:(n + 1) * 512], ps[:])
                    nc.sync.dma_start(
                        y[P * 128:(P + 1) * 128,
                          dms[0] * 512:(dms[-1] + 1) * 512], yt[:])
                return emit

            from collections import deque
            fillers = deque()   # (key, emit_fn, req); keys track emission
            chainq = deque()    # deferred normalization-chain ops
            emitted = set()
            chain_emitted = [0]

            def pop_chain():
                chainq.popleft()()
                chain_emitted[0] += 1

            def pop_filler():
                key, fn, req = fillers[0]
                if key in emitted:
                    fillers.popleft()
                    return
                # a filler may read tiles written by deferred chain ops;
                # force-emit the chain up to its snapshot first
                while chain_emitted[0] < req and chainq:
                    pop_chain()
                fillers.popleft()
                fn()
                emitted.add(key)
                units_left[0] -= 1

            def drain_until(keys):
                # engines run their queues in emission order, so a unit
                # producing data for block (pri, i) must be EMITTED before
                # the block's first consumer instruction
                while fillers and not keys <= emitted:
                    pop_filler()

            # preamble compute: v tiles 0..3 + pair-0 q/k block 0.
            # v0..3 are emitted as m-halves over four simultaneous psum
            # groups (2 proj bufs + 2 borrowed ev bufs) so their m0..3
            # matmuls run as soon as the first two x blobs land, ~10us
            # before the second pair arrives.
            pre_t = [pp.tile([128, 512], f32, tag="proj", bufs=2,
                             name=f"pre{p}") for p in range(2)]
            pre_ev = [pp.tile([128, 1024], f32, tag="ev", bufs=2,
                              name=f"prev{p}") for p in range(2)]
            pre_ps = [pre_t[0][:], pre_t[1][:],
                      pre_ev[0][:, 0:512], pre_ev[1][:, 0:512]]
            for p in range(4):
                for m in range(4):
                    nc.tensor.matmul(
                        pre_ps[p], xsl(m, p * 128, (p + 1) * 128),
                        wv_sb[:, m * 512:(m + 1) * 512],
                        start=(m == 0), stop=False)
            for p in range(4):
                for m in range(4, MT):
                    nc.tensor.matmul(
                        pre_ps[p], xsl(m, p * 128, (p + 1) * 128),
                        wv_sb[:, m * 512:(m + 1) * 512],
                        start=False, stop=(m == MT - 1))
                vproj_evac(p, pre_ps[p])
                emitted.add(("v", p))
            unit_qkproj(0, 0, 0)()
            unit_qkproj(1, 0, 0)()
            emitted.update({("q", 0, 0), ("k", 0, 0)})
            # q/k block 0 for the other pairs (i=0 needs them), then the
            # tiles later i-phases consume, interleaved in dependency order
            for pri in range(1, NPAIR):
                fillers.append((("q", pri, 0), unit_qkproj(0, pri, 0), 0))
                fillers.append((("k", pri, 0), unit_qkproj(1, pri, 0), 0))
            fillers.extend((("v", p), unit_vproj(p), 0) for p in (4, 5))
            for pri in range(NPAIR):
                fillers.append((("q", pri, 1), unit_qkproj(0, pri, 1), 0))
                fillers.append((("k", pri, 1), unit_qkproj(1, pri, 1), 0))
            fillers.extend((("v", p), unit_vproj(p), 0) for p in (6, 7))

            total_j = NPAIR * sum(4 * (i2 + 1) for i2 in range(NQP))
            done_j = [0]
            # total filler units over the whole schedule: 16 vproj + 32
            # qkproj + 16 oproj, minus the 6 emitted in the preamble
            units_left = [16 + 32 + 16 - 6]

            def block_keys(pri, i):
                return ({("q", pri, i)}
                        | {("k", pri, pb) for pb in range(i + 1)}
                        | {("v", p) for p in range(4 * (i + 1))})

            def mk_logits(pri, i):
                qT = qkT[("q", pri)]
                kT = qkT[("k", pri)]

                def emit_logits(j):
                    o = (j - 4 * i) * 128 if j >= 4 * i else 0
                    ev = pp.tile([128, 1024], f32, tag="ev", bufs=2,
                                 name="ev")
                    for h in range(2):
                        nc.tensor.matmul(
                            ev[:, h * 512 + o:(h + 1) * 512],
                            kT[64 * h:64 * h + 64, j * 128:(j + 1) * 128],
                            qT[64 * h:64 * h + 64,
                               i * 512 + o:(i + 1) * 512],
                            start=True, stop=True)
                    return ev, o
                return emit_logits

            pending_ev = {}   # (pri, i) -> {j: (ev, o)} primed cross-block

            def emit_block(pri, i, nxt=None):
                kmax = 4 * (i + 1)
                rem_j = total_j - done_j[0]
                stride = max(1, rem_j // max(1, units_left[0]))
                drain_until(block_keys(pri, i))
                for _ in range(2 if len(fillers) > 5 else 1):
                    if fillers:
                        pop_filler()
                ad = pp.tile([65, 1024], f32, tag="ad", bufs=1)
                emit_logits = mk_logits(pri, i)

                # logits run two j's ahead so the ACT exp stream never
                # starves across interleaved filler matmuls; the first
                # one or two may have been primed by the previous block
                evq = pending_ev.pop((pri, i), {})
                if 0 not in evq:
                    evq[0] = emit_logits(0)
                if kmax > 1 and 1 not in evq:
                    evq[1] = emit_logits(1)
                nxt_logits = mk_logits(*nxt) if nxt is not None else None
                for j in range(kmax):
                    ev, o = evq.pop(j)
                    sc = sp.tile([128, 1024], bf16, tag="sc", bufs=8)
                    if o:
                        nc.scalar.activation(
                            sc.rearrange("p (h c) -> p h c",
                                         c=512)[:, :, o:],
                            ev.rearrange("p (h c) -> p h c",
                                         c=512)[:, :, o:],
                            Exp, scale=0.125)
                    else:
                        nc.scalar.activation(sc[:], ev[:], Exp,
                                             scale=0.125)
                    if j >= 4 * i:
                        # staircase mask on the 128-wide diagonal strip
                        # (both heads in one DVE multiply)
                        strip = sc.rearrange(
                            "p (h c) -> p h c", c=512)[:, :, o:o + 128]
                        nc.vector.tensor_mul(
                            strip, strip,
                            stair_sb.rearrange("p (h c) -> p h c", c=128))
                    st = (j == 0)
                    sp_ = (j == kmax - 1)
                    vt = v_sb[j]
                    for h in range(2):
                        lh = 2 * pri + h
                        nc.tensor.matmul(
                            ad[0:65, h * 512 + o:(h + 1) * 512],
                            vt[:, lh * 65:lh * 65 + 65],
                            sc[:, h * 512 + o:(h + 1) * 512],
                            start=st, stop=sp_, skip_group_check=True)
                    if j + 2 < kmax:
                        evq[j + 2] = emit_logits(j + 2)
                    elif nxt is not None and j + 2 - kmax <= 1:
                        # prime the next block's first logits in the ev
                        # slots this block no longer needs, so its exp's
                        # are already done when its j-loop starts
                        jn = j + 2 - kmax
                        if jn == 0:
                            drain_until(block_keys(*nxt))
                        pending_ev.setdefault(nxt, {})[jn] = nxt_logits(jn)
                    if chainq:
                        pop_chain()
                    if fillers and (j % stride == stride - 1):
                        pop_filler()
                done_j[0] += kmax
                # normalization: evacuate ad via DVE (releases the PSUM
                # bank fast); the rest of the chain (DVE reciprocal ->
                # gpsimd partition broadcast -> two DVE multiplies -> DMA
                # repack) is deferred into the next block's j-loop so it
                # never head-of-line-blocks its engine queue.
                adc = sp.tile([65, 1024], f32, tag="adc", bufs=4,
                              name="adc")
                nc.vector.tensor_copy(adc[:], ad[:, :])
                at = at_sb[(pri, i)]
                tmp = sp.tile([64, 512], bf16, tag="tmp", bufs=4)
                lnt = sp.tile([1, 1024], f32, tag="lnt", bufs=2,
                              name="lnt")
                rec = sp.tile([1, 1024], f32r, tag="rec", bufs=2,
                              name="rec")

                def chain_ops(adc=adc, at=at, tmp=tmp, rec=rec, lnt=lnt):
                    # 1/den as exp(-ln(den)) on ACT (same table set as
                    # the softmax exp), then a PE broadcast matmul
                    # (one64 x rec) replicates it across 64 partitions
                    bch = [pp.tile([128, 512], f32, tag="proj", bufs=2,
                                   name=f"bc{hh}") for hh in range(2)]

                    def bc_mm(hh):
                        return lambda: nc.tensor.matmul(
                            bch[hh][0:64, :], one64_sb[:],
                            rec[:, hh * 512:(hh + 1) * 512],
                            start=True, stop=True)
                    yield lambda: nc.scalar.activation(
                        lnt[:], adc[64:65, :], Ln)
                    yield lambda: nc.scalar.activation(
                        rec[:], lnt[:], Exp, scale=-1.0)
                    yield bc_mm(1)
                    yield bc_mm(0)
                    yield lambda: nc.vector.tensor_mul(
                        tmp[:], adc[0:64, 512:1024], bch[1][0:64, :])
                    yield lambda: nc.sync.dma_start(at[64:128, :],
                                                    tmp[:])
                    yield lambda: nc.vector.tensor_mul(
                        at[0:64, :], adc[0:64, 0:512], bch[0][0:64, :])

                if nxt is None:
                    # last block: no later j-loop will pop these; run the
                    # chain immediately so the final O units aren't stuck
                    # behind a fully serialized Ln/Exp/bc/mul/DMA chain
                    for op in chain_ops():
                        op()
                        chain_emitted[0] += 1
                else:
                    chainq.extend(chain_ops())

            # ---- attention: i-outer, pair-inner ----
            seq = [(pri, i) for i in range(NQP) for pri in range(NPAIR)]
            nxt_of = {seq[n]: seq[n + 1] for n in range(len(seq) - 1)}
            for i in range(NQP):
                for pri in range(NPAIR):
                    emit_block(pri, i, nxt_of.get((pri, i)))
                # O-projection units for block i become fillers for the
                # next phase (gated on all four pairs' chains finishing)
                req = chain_emitted[0] + len(chainq)
                if i == NQP - 1:
                    # split per dm-half at the end: finer tail overlap
                    for pt in range(4):
                        for dm in range(2):
                            fillers.append((("o", i, pt, dm),
                                            unit_oproj(i, pt, (dm,)), req))
                else:
                    for pt in range(4):
                        fillers.append((("o", i, pt), unit_oproj(i, pt),
                                        req))
                # stage the tiles phase i+2 consumes behind the O units
                # (the preamble already queued everything phases 0-1 need)
                if i + 2 < NQP:
                    queued = emitted | {f[0] for f in fillers}
                    for pri in range(NPAIR):
                        for key, ti in ((("q", pri, i + 2), 0),
                                        (("k", pri, i + 2), 1)):
                            if key not in queued:
                                fillers.append(
                                    (key, unit_qkproj(ti, pri, i + 2), 0))
                    fillers.extend(
                        (("v", p), unit_vproj(p), 0)
                        for p in range(4 * (i + 2), min(4 * (i + 3), NKT))
                        if ("v", p) not in queued)

            while chainq:
                pop_chain()
            while fillers:
                pop_filler()

    _split_multi_waits(nc, mybir)
    _cache["nc"] = nc
    return nc


def _host_inputs(x, Q_w, Q_b, K_w, K_b, V_w, V_b, O_w):
    import ml_dtypes
    bf = ml_dtypes.bfloat16
    stair = (np.arange(128)[:, None] <= np.arange(128)[None, :]).astype(bf)
    stair2 = np.concatenate([stair, stair], axis=1)
    in_maps = []
    for c in range(8):
        b, hs = c // 2, HPC * (c % 2)
        he = hs + HPC
        qb = Q_b[hs:he].reshape(512).astype(np.float32)
        kb = K_b[hs:he].reshape(512).astype(np.float32)
        qkb = np.zeros((128, 8), np.float32)
        for pri in range(NPAIR):
            qkb[:, pri] = qb[pri * 128:(pri + 1) * 128]
            qkb[:, 4 + pri] = kb[pri * 128:(pri + 1) * 128]
        xTb = np.ascontiguousarray(x[b].T).astype(bf)       # [DM, S]
        # weight blobs: [128, MT*512] with column block m = m-tile
        def wblob(W):  # W: [H/2==8 heads? no: [heads, DM, DH]] slice
            w2d = W[hs:he].transpose(1, 0, 2).reshape(DM, 512).astype(bf)
            return np.ascontiguousarray(
                w2d.reshape(MT, 128, 512).transpose(1, 0, 2).reshape(
                    128, MT * 512))
        wo2d = O_w[hs:he].reshape(512, DM).astype(bf)
        in_maps.append({
            **{f"xP{g}": np.ascontiguousarray(
                np.concatenate([xTb[2 * g * 128:(2 * g + 1) * 128, :],
                                xTb[(2 * g + 1) * 128:(2 * g + 2) * 128, :]],
                               axis=1)) for g in range(MT // 2)},
            "Wq": wblob(Q_w),
            "Wk": wblob(K_w),
            "Wv": wblob(V_w),
            "Wo": np.ascontiguousarray(
                wo2d.reshape(4, 128, DM).transpose(1, 0, 2).reshape(
                    128, 4 * DM)),
            "qkb": qkb,
            "vbb": np.tile(V_b[hs:he].reshape(1, 512), (128, 1)).astype(bf),
            "stair2": stair2,
            "onz": np.ones((128, 8), bf),
            "one64": np.ones((1, 64), np.float32),
        })
    return in_maps


def kernel(x, Q_w, Q_b, K_w, K_b, V_w, V_b, O_w, O_b, _trace=False):
    x = np.asarray(x, np.float32)
    args = [np.asarray(a, np.float32)
            for a in (Q_w, Q_b, K_w, K_b, V_w, V_b, O_w)]
    O_b = np.asarray(O_b, np.float32)

    nc = _build()
    from concourse.bass_utils import run_bass_kernel_spmd

    in_maps = _host_inputs(x, *args)
    res = run_bass_kernel_spmd(nc, in_maps, core_ids=list(range(8)),
                               trace=_trace)
    _cache["last_result"] = res
    out = np.empty((B, S, DM), np.float32)
    for b in range(B):
        out[b] = (res.results[2 * b]["y"].astype(np.float32)
                  + res.results[2 * b + 1]["y"].astype(np.float32) + O_b)
    return out


if __name__ == "__main__":
    # quick self-run with random inputs
    rng = np.random.default_rng(0)
    x = rng.standard_normal((B, S, DM), dtype=np.float32)
    shp = dict(Q_w=(H, DM, DH), Q_b=(H, DH), K_w=(H, DM, DH), K_b=(H, DH),
               V_w=(H, DM, DH), V_b=(H, DH), O_w=(H, DH, DM), O_b=(DM,))
    ins = {k: rng.standard_normal(v, dtype=np.float32) * 0.05
           for k, v in shp.items()}
    out = kernel(x, **ins)
    print("ran", out.shape, out.dtype)
